# revision 82
# baseline (speedup 1.0000x reference)
"""Causal self-attention (B=2, N=2048, D=1024, H=16, hd=64) on 8 trn2 NeuronCores.

Sharding: core c handles batch b = c//4 and 4 heads hs = [4*(c%4) .. 4*(c%4)+3]
(tensor-parallel over heads x data-parallel over batch). Each core computes its
heads' attention and a row-parallel partial of the output projection; the host
sums the 4 partials per batch and adds the output bias.

v3: mixed-precision fp8/bf16 device algorithm (cost model 93.8us).
  - qk projection: fp8e4m3 DoubleRow matmuls (2x128 contraction per
    instruction) producing qkT8 in a "quad32" layout: head h occupies
    partitions [32h, 32h+32); free dim groups {k-half0, k-half1, q-half0,
    q-half1} so score matmuls contract the 64-dim head via [32, 2] DoubleRow.
  - scores: fp8 DoubleRow, one matmul per (head, key-tile).
  - attn@v: off-diagonal key-tile PAIRS via fp8 DoubleRow over v128_8;
    diagonal tiles in bf16 (accurate v for high-weight keys). Each head's
    v block carries 64 v columns + 64 ONES columns, so the AV matmul emits
    the softmax denominator replicated on PSUM rows 64:128 — normalization
    is one DVE reciprocal off those rows + one DVE multiply (no Pool
    partition_broadcast).
  - exp on ScalarE (scale=1/8 folded; the Act engine is the throughput
    wall at ~84% busy), causal tri-mask multiplied into the bf16 at tiles
    on the Pool engine. Diag tiles d2+d3 share one PSUM tile and one exp
    instruction, packed by HEAD per bank (a PSUM bank written from two PE
    tile_positions faults the device).
  - rows 0..127 (few-key softmax rows, precision critical) recomputed
    exactly in bf16 by a small "protection" path that overwrites
    saT[:, :, 0:128].
  - output projection row-parallel in fp32r; partials shipped bf16 and
    summed f32 on the host.
  - startup: a back-to-back warm-up matmul chain ramps the PE p-state
    through the DMA window (full clock by qk_proj(0)); block-0 DMAs and
    ct order are arranged so the first exp lands ~6.4us; block-0 qk
    evacuations run on the otherwise-idle Act engine (Identity +
    per-partition bias AP).
  - tail: the last q-block's AV/normalize/out-project/DMA pipeline is
    chunked per 128 q-columns with copies split Act/DVE.
"""

import numpy as np
import ml_dtypes
from contextlib import ExitStack

import concourse.bass as bass
import concourse.tile as tile
from concourse import bacc, mybir
from concourse import bass_utils

F32 = mybir.dt.float32
F32R = mybir.dt.float32r
BF16 = mybir.dt.bfloat16
FP8 = mybir.dt.float8e4
I16 = mybir.dt.int16
EXP = mybir.ActivationFunctionType.Exp
DR = mybir.MatmulPerfMode.DoubleRow
ALU = mybir.AluOpType

# Schraudolph exp-approximation for DVE offload: bf16 bits of exp(x/8) are
# int16(x * SCH_A + SCH_B) (rint rounding, tuned B; max rel err 3.3%).
SCH_A = 184.6650526 / 8.0
SCH_B = 16250.375

B, N, D, H, HD = 2, 2048, 1024, 16, 64
N_CORES = 8
LH = 4            # local heads per core
KT = D // 128     # 8 contraction k-tiles
NT = N // 128     # 16 n-tiles
NB = N // 512     # 4 n-blocks / q-blocks
QB = 512

_CACHE: dict = {}

# tuning knobs (A/B experiments)
CFG = {
    "at8_bufs": 14,
    "at16_bufs": 10,
    "oe_bufs": 4,
    "sc_bufs": 2,
    "norm_engine": "dve",    # gpsimd cannot read PSUM
    "tri_engine": "pool",    # at16 mask engine: pool | dve
    "psum_dma": False,       # DMA cannot read PSUM directly
    "protect": True,
    "act_primer": True,
    # exp offload: which diag tiles (J, p) -> set of d run Schraudolph on DVE
    "sch_diag": {},
    # off-diag pairs (J, p) -> set of m run Schraudolph on DVE (bf16 AV)
    "sch_off": {},
    "at16o_bufs": 6,
    # oe evacuation engine per (J, q): "act" entries
    "oe_act": set(),
    # qk evac engine per (nb, ct): "act" entries (Identity + per-partition bias)
    "qk_act": set(),
    # ship output partials as bf16 (halves the out DMA; host sums in f32)
    "out_bf16": True,
    "skip_attention": False,
    "skip_outproj": False,
    "skip_vproj": False,
    "skip_qkproj": False,
    "skip_av": False,
    "skip_norm": False,
    "skip_protattn": False,
    "skip_protav": False,
    "skip_protnorm": False,
}
import os as _os
for _k in ("skip_attention", "skip_outproj", "skip_vproj", "skip_qkproj", "protect",
           "skip_av", "skip_norm", "skip_protattn", "skip_protav", "skip_protnorm"):
    _v = _os.environ.get("K_" + _k.upper())
    if _v is not None:
        CFG[_k] = _v == "1"


def _emit(nc, tc, ctx, io, repeat=1):
    (xT8, xT16, wqk8, wqk16, wv, bqk8, bqk16, bv, wo, tri16, out) = io

    persist = ctx.enter_context(tc.tile_pool(name="persist", bufs=1))
    sbp = ctx.enter_context(tc.tile_pool(name="work", bufs=1))
    psum = ctx.enter_context(tc.tile_pool(name="psum", bufs=1, space="PSUM"))

    # ---- persistent SBUF tensors ----
    xT8_sb = persist.tile([128, KT, N], FP8)
    xT16_sb = persist.tile([128, KT, N], BF16)
    wqk8_sb = persist.tile([128, KT, 512], FP8)
    wqk16_sb = persist.tile([128, KT, 512], BF16)
    wv_sb = persist.tile([128, KT, 256], BF16)
    wo_sb = persist.tile([128, 2, 1024], F32R)
    bqk8_sb = persist.tile([128, 4], F32)
    bqk16_sb = persist.tile([128, 4], F32)
    bv_sb = persist.tile([1, 256], F32R)
    ones_sb = persist.tile([1, 128], F32R)
    warm_sb = persist.tile([1, 512], F32R)
    tri16_sb = persist.tile([128, 128], BF16)
    # [p, {kh0,kh1,qh0,qh1}, n]; head h on partitions [32h, 32h+32)
    qkT8_sb = persist.tile([128, 4, N], FP8)
    # protection block qkT, bf16. Groups {kp, q-z0, q-z1} per pair: the q
    # copies are zero-padded on the other head's partitions so score matmuls
    # can contract the full 128 partitions (device faults on non-DoubleRow
    # matmuls at 32/64-row tile positions when DoubleRow is also in use).
    qkTp_sb = persist.tile([128, 6, 128], BF16)
    # v128: per head 128 cols = {64 v-dims, 64 ones}. The ones block makes
    # the AV matmul emit the softmax denominator REPLICATED on out rows
    # 64:128, so normalization needs no partition_broadcast: one DVE
    # reciprocal off those rows (same free-size cost as a [1,512] recip)
    # and one DVE multiply. (TT divide is not a valid DVE op on TRN2.)
    v128_16 = persist.tile([128, NT, LH * 128], BF16)
    v128_8 = persist.tile([128, NT, LH * 136], FP8)  # 136-stride: dual-fp8 16B pair step
    saT_sb = persist.tile([128, 2, N], F32R)

    # warm-up chain first: its memset must not queue behind other DVE work,
    # and the matmuls must run back-to-back through the DMA window so the PE
    # p-state is fully ramped (~3us continuous) when qk_proj(0) starts.
    warm2 = persist.tile([128, 512], BF16)
    nc.vector.memset(warm2[:], 0.0)
    for _w in range(CFG.get("warm_mms", 7)):
        wp = psum.tile([128, 512], F32, name="ps_op", tag="op", bufs=2)
        nc.tensor.matmul(wp[:], warm2[:, 0:128], warm2[:, :],
                         start=True, stop=True)

    # ---- input DMAs, batched (HWDGE dispatch ~625ns each). The fp8 x blocks
    # all come first: they feed the score pipeline that keeps the Act engine
    # (the throughput wall) busy; bf16 x / prot weights can land later ----
    xT16v = xT16.rearrange("(t p) n -> p t n", t=KT)
    xT8v = xT8.rearrange("(t p) n -> p t n", t=KT)
    # first-exp critical chain: x block 0 and the q-side weight cols (ct2/3,
    # cols 256:512) land first; qk_proj(0) runs ct order (2,3,0,1) to match
    wqk8v = wqk8.rearrange("(t p) c -> p t c", t=KT)
    nc.sync.dma_start(xT8_sb[:, 0:4, 0:QB], xT8v[:, 0:4, 0:QB])
    nc.sync.dma_start(xT8_sb[:, 4:8, 0:QB], xT8v[:, 4:8, 0:QB])
    nc.sync.dma_start(wqk8_sb[:, :, 256:512], wqk8v[:, :, 256:512])
    nc.sync.dma_start(bqk8_sb[:], bqk8.rearrange("t p -> p t"))
    nc.sync.dma_start(wqk8_sb[:, :, 0:256], wqk8v[:, :, 0:256])
    nc.sync.dma_start(xT8_sb[:, :, QB:2 * QB], xT8v[:, :, QB:2 * QB])
    nc.sync.dma_start(bqk16_sb[:], bqk16.rearrange("t p -> p t"))
    nc.sync.dma_start(bv_sb[:], bv[:])
    nc.sync.dma_start(tri16_sb[:], tri16[:])
    nc.sync.dma_start(wv_sb[:], wv.rearrange("(t p) c -> p t c", t=KT))
    nc.sync.dma_start(xT16_sb[:, :, 0:QB], xT16v[:, :, 0:QB])
    for nb in range(2, NB):
        nc.sync.dma_start(xT8_sb[:, :, nb * QB:(nb + 1) * QB],
                          xT8v[:, :, nb * QB:(nb + 1) * QB])
    nc.sync.dma_start(wqk16_sb[:], wqk16.rearrange("(t p) c -> p t c", t=KT))
    for nb in range(1, NB):
        nc.sync.dma_start(xT16_sb[:, :, nb * QB:(nb + 1) * QB],
                          xT16v[:, :, nb * QB:(nb + 1) * QB])
    nc.sync.dma_start(wo_sb[:], wo.rearrange("(t p) c -> p t c", t=2))
    nc.vector.memset(ones_sb[:].bitcast(F32), 1.0)
    nc.vector.memset(warm_sb[:].bitcast(F32), 1.0)
    # only the ones-columns need init; v copies fill the rest. Pool engine
    # (memset efficiency 1.0) during the initial DMA window.
    nc.gpsimd.memset(
        v128_16[:, :, :].rearrange("p t (h c) -> p t h c", c=128)[:, :, :, 64:128], 1.0)
    nc.gpsimd.memset(
        v128_8[:, :, :].rearrange("p t (h c) -> p t h c", c=136)[:, :, :, 64:128], 1.0)
    if CFG.get("act_primer", True):
        # load the exp table set before the first real activation needs it
        primer = sbp.tile([1, 1], F32, name="t_primer", tag="primer", bufs=1)
        nc.scalar.activation(primer[:], ones_sb[0:1, 0:1].bitcast(F32), EXP)

    # ---- phase helpers ----
    def v_proj(nt):
        if CFG["skip_vproj"]:
            return
        ps = psum.tile([128, 512], F32, name="ps_pj", tag="op", bufs=2)[:, 0:256]
        for kt in range(KT):
            nc.tensor.matmul(
                ps[:], xT16_sb[:, kt, nt * 128:(nt + 1) * 128], wv_sb[:, kt, :],
                start=(kt == 0), stop=False,
            )
        nc.tensor.matmul(ps[:], ones_sb[:], bv_sb[:], start=False, stop=True)
        src = ps[:, :].rearrange("p (h c) -> p h c", c=64)
        v16 = v128_16[:, nt, :].rearrange("p (h c) -> p h c", c=128)[:, :, 0:64]
        nc.vector.tensor_copy(v16, src)
        # fp8 copy derived from the bf16 one (gpsimd cannot read PSUM)
        v8e = nc.gpsimd if CFG.get("v8_engine", "pool") == "pool" else nc.vector
        v8e.tensor_copy(
            v128_8[:, nt, :].rearrange("p (h c) -> p h c", c=136)[:, :, 0:64], v16)

    QK0 = {}

    def _qk0_evac(nb, i, ct, a, b):
        dst = qkT8_sb[:, ct, nb * QB + a:nb * QB + b]
        src = QK0[ct][:, a:b]
        if i in CFG.get("qk0_act_idx", {0, 3}):
            nc.scalar.activation(
                dst, src, mybir.ActivationFunctionType.Identity,
                bias=bqk8_sb[:, ct:ct + 1])
        else:
            nc.vector.tensor_scalar_add(dst, src, bqk8_sb[:, ct:ct + 1])

    def qk_proj(nb, phase=None):
        if CFG["skip_qkproj"]:
            return
        if phase == "b":
            # k-side full-width evacs; DVE by default, in parallel with the
            # first diag exp on Act
            _qk0_evac(nb, 2, 0, 128, 512)
            _qk0_evac(nb, 3, 1, 128, 512)
            return
        for ct in (2, 3, 0, 1):
            ps = psum.tile([128, 1024], F32, name="ps_qk0", tag="sc", bufs=2)[:, 0:512]
            QK0[ct] = ps
            for j in range(4):
                nc.tensor.matmul(
                    ps[:], wqk8_sb[:, 2 * j:2 * j + 2, ct * 128:(ct + 1) * 128],
                    xT8_sb[:, 2 * j:2 * j + 2, nb * QB:(nb + 1) * QB],
                    start=(j == 0), stop=(j == 3), perf_mode=DR,
                )
            if ct == 1:
                for c in range(2):
                    nc.vector.tensor_scalar_add(
                        qkT8_sb[:, c, nb * QB:nb * QB + 128], QK0[c][:, 0:128],
                        bqk8_sb[:, c:c + 1])
        # q-side evacs on Act (idle until the first exp); the d0 diag scores
        # need only these plus the early 0:128 k evacs
        _qk0_evac(nb, 0, 2, 128, 512)
        _qk0_evac(nb, 1, 3, 128, 512)
        if not CFG["protect"]:
            # q cols 0:128 must be evac'd BEFORE ct0/ct1 matmuls recycle the
            # ct2/ct3 psum buffers... emitted here they still race; but the
            # tile framework orders the reads before the WAR overwrite
            _qk0_evac(nb, 4, 2, 0, 128)
            _qk0_evac(nb, 5, 3, 0, 128)

    def prot_qk():
        if CFG["skip_protattn"]:
            return
        nc.vector.memset(qkTp_sb[:], 0.0)
        for ct in range(4):
            pp = psum.tile([128, 512], F32, name="ps_pj", tag="op", bufs=2)[:, 0:128]
            for kt in range(KT):
                nc.tensor.matmul(
                    pp[:], wqk16_sb[:, kt, ct * 128:(ct + 1) * 128],
                    xT16_sb[:, kt, 0:128],
                    start=(kt == 0), stop=(kt == KT - 1),
                )
            pr, is_q = divmod(ct, 2)
            if not is_q:
                nc.vector.tensor_scalar_add(
                    qkTp_sb[:, 3 * pr, :], pp[:], bqk16_sb[:, ct:ct + 1])
            else:
                # q copies zero-padded per head: head s only on its own 64
                # partitions so the score matmul can contract all 128 rows
                nc.vector.tensor_scalar_add(
                    qkTp_sb[0:64, 3 * pr + 1, :], pp[0:64, :],
                    bqk16_sb[0:64, ct:ct + 1])
                nc.vector.tensor_scalar_add(
                    qkTp_sb[64:128, 3 * pr + 2, :], pp[64:128, :],
                    bqk16_sb[64:128, ct:ct + 1])

    def prot_attn():
        if CFG["skip_protattn"]:
            return
        accp = psum.tile([128, 512], F32, name="ps_acc", tag="acc", bufs=2)
        for p in range(2):
            ps2 = psum.tile([128, 512], F32, name="ps_pj", tag="op", bufs=2)[:, 0:256]
            for s in range(2):
                # full-128-row matmul; the other head's q partitions are zero
                nc.tensor.matmul(
                    ps2[:, s * 128:(s + 1) * 128],
                    qkTp_sb[:, 3 * p, :],
                    qkTp_sb[:, 3 * p + 1 + s, :],
                    start=True, stop=True,
                )
            atp = sbp.tile([128, 256], BF16, name="t_atp", tag="atp", bufs=2)
            nc.scalar.activation(atp[:], ps2[:], EXP, scale=0.125)
            atv = atp[:].rearrange("p (s c) -> p s c", c=128)
            nc.vector.tensor_mul(
                atv, atv, tri16_sb[:, None, :].broadcast_to([128, 2, 128]))
            for s in range(2):
                if CFG["skip_protav"]:
                    break
                h = 2 * p + s
                nc.tensor.matmul(
                    accp[0:128, h * 128:(h + 1) * 128],
                    v128_16[:, 0, h * 128:h * 128 + 128],
                    atp[:, s * 128:(s + 1) * 128],
                    start=True, stop=True, skip_group_check=True,
                )
        for h in range(LH):
            if CFG["skip_protav"] or CFG["skip_protnorm"]:
                break
            dn = sbp.tile([64, 512], F32, name="t_dn", tag="dn", bufs=4)[:, 0:128]
            nc.vector.reciprocal(dn[:], accp[64:128, h * 128:(h + 1) * 128])
            po = (h % 2) * 64
            nc.vector.tensor_mul(
                saT_sb[po:po + 64, h // 2, 0:128],
                accp[0:64, h * 128:(h + 1) * 128], dn[:])

    ATT = {}  # (J, p) -> dict(at8=[...], at16=[...])

    def _weave(wv):
        if wv:
            u = wv.pop(0)
            u()

    def att_scores(J, p, weave=None, diag_first=False, ds=None):
        if CFG["skip_attention"] or CFG["skip_vproj"] or CFG["skip_qkproj"]:
            return
        if (J, p) in ATT and ds is not None:
            # continuation: emit only the remaining diag tiles
            _diag_scores(J, p, ATT[(J, p)], weave, ds=ds)
            return
        q_lo = 128 if (J == 0 and CFG["protect"]) else 0
        st = {"at8": [], "at16": [], "q_lo": q_lo, "diag_first": diag_first}
        ATT[(J, p)] = st
        if ds is not None:
            _diag_scores(J, p, st, weave, ds=ds)
            return
        if diag_first:
            _diag_scores(J, p, st, weave)
        for m in range(2 * J):
            sch = m in CFG["sch_off"].get((J, p), ())
            if sch:
                at8 = sbp.tile([128, 2, 1024], BF16, name="t_at16o",
                               tag="at16o", bufs=CFG["at16o_bufs"])
            else:
                at8 = sbp.tile([128, 2, 1024], FP8, name="t_at8", tag="at8",
                               bufs=CFG["at8_bufs"])
            st["at8"].append((at8, sch))
            for half in range(2):
                t = 2 * m + half
                sc = psum.tile([128, 1024], F32, name="ps_sc", tag="sc",
                               bufs=CFG["sc_bufs"])
                for s in range(2):
                    h = 2 * p + s
                    nc.tensor.matmul(
                        sc[:, s * 512:(s + 1) * 512],
                        qkT8_sb[32 * h:32 * h + 32, 0:2, t * 128:(t + 1) * 128],
                        qkT8_sb[32 * h:32 * h + 32, 2:4, J * QB:(J + 1) * QB],
                        start=True, stop=True, perf_mode=DR,
                        tile_position=(32 * h, 0),
                    )
                if sch:
                    nc.vector.tensor_scalar(
                        at8[:, half, :].bitcast(I16), sc[:],
                        SCH_A, SCH_B, ALU.mult, ALU.add)
                else:
                    nc.scalar.activation(at8[:, half, :], sc[:], EXP, scale=0.125)
            _weave(weave)
        if not diag_first:
            _diag_scores(J, p, st, weave)

    def _diag_scores(J, p, st, weave, ds=None):
        q_lo = st["q_lo"]
        ds = list(range(4)) if ds is None else list(ds)
        merge23 = CFG.get("merge_d23", True) and 2 in ds and 3 in ds
        for d in ds:
            t = 4 * J + d
            c0 = max(d * 128, q_lo)
            if merge23 and d == 3:
                continue  # emitted together with d == 2
            if merge23 and d == 2:
                # d2 (2 heads x 256 q) and d3 (2 heads x 128 q) packed into
                # ONE psum tile and ONE exp instruction. Grouped by HEAD so
                # each PSUM bank is written from a single PE tile_position
                # (two positions into one bank faults the device): bank s
                # holds [d2s (0:256) | d3s (256:384)].
                c0b = max(384, q_lo)
                w2, w3 = 512 - c0, 512 - c0b
                sc = psum.tile([128, 1024], F32, name="ps_sc", tag="sc",
                               bufs=CFG["sc_bufs"])
                for s in range(2):
                    h = 2 * p + s
                    nc.tensor.matmul(
                        sc[:, s * 512:s * 512 + w2],
                        qkT8_sb[32 * h:32 * h + 32, 0:2, t * 128:(t + 1) * 128],
                        qkT8_sb[32 * h:32 * h + 32, 2:4, J * QB + c0:(J + 1) * QB],
                        start=True, stop=True, perf_mode=DR,
                        tile_position=(32 * h, 0),
                    )
                    nc.tensor.matmul(
                        sc[:, s * 512 + w2:s * 512 + w2 + w3],
                        qkT8_sb[32 * h:32 * h + 32, 0:2, (t + 1) * 128:(t + 2) * 128],
                        qkT8_sb[32 * h:32 * h + 32, 2:4, J * QB + c0b:(J + 1) * QB],
                        start=True, stop=True, perf_mode=DR,
                        tile_position=(32 * h, 0),
                    )
                at16m = sbp.tile([128, 2, 512], BF16, name="t_at16m", tag="at16",
                                 bufs=CFG["at16_bufs"])
                st["at16"].append(("packed", at16m, c0, 0, w2))
                st["at16"].append(("packed", at16m, c0b, w2, w3))
                scv2 = sc[:, :].rearrange("p (s c) -> p s c", c=512)
                nc.scalar.activation(at16m[:, :, 0:w2 + w3], scv2[:, :, 0:w2 + w3],
                                     EXP, scale=0.125)
                trie = nc.gpsimd if CFG["tri_engine"] == "pool" else nc.vector
                # d2 diagonal block: first 128 cols of its range
                a2 = at16m[:, :, 0:128]
                trie.tensor_mul(
                    a2, a2, tri16_sb[:, None, :].broadcast_to([128, 2, 128]))
                # d3 diagonal block: its whole 128-wide range
                a3 = at16m[:, :, w2:w2 + 128]
                trie.tensor_mul(
                    a3, a3, tri16_sb[:, None, :].broadcast_to([128, 2, 128]))
                _weave(weave)
                continue
            sc = psum.tile([128, 1024], F32, name="ps_sc", tag="sc",
                           bufs=CFG["sc_bufs"])
            for s in range(2):
                h = 2 * p + s
                nc.tensor.matmul(
                    sc[:, s * 512 + c0:(s + 1) * 512],
                    qkT8_sb[32 * h:32 * h + 32, 0:2, t * 128:(t + 1) * 128],
                    qkT8_sb[32 * h:32 * h + 32, 2:4, J * QB + c0:(J + 1) * QB],
                    start=True, stop=True, perf_mode=DR,
                    tile_position=(32 * h, 0),
                )
            at16 = sbp.tile([128, 2, 512], BF16, name="t_at16", tag="at16",
                            bufs=CFG["at16_bufs"])
            st["at16"].append(("old", at16, c0, 0, 0))
            scv = sc[:, :].rearrange("p (s c) -> p s c", c=512)
            if d in CFG["sch_diag"].get((J, p), ()):
                nc.vector.tensor_scalar(
                    at16[:, :, c0:512].bitcast(I16), scv[:, :, c0:512],
                    SCH_A, SCH_B, ALU.mult, ALU.add)
            else:
                nc.scalar.activation(at16[:, :, c0:512], scv[:, :, c0:512],
                                     EXP, scale=0.125)
            cm = d * 128
            if cm >= q_lo:
                # causal tri-mask on the diagonal 128-block of both heads
                atv = at16[:, :, cm:cm + 128]
                trie = nc.gpsimd if CFG["tri_engine"] == "pool" else nc.vector
                trie.tensor_mul(
                    atv, atv, tri16_sb[:, None, :].broadcast_to([128, 2, 128]))
            _weave(weave)

    def _diag_av(J, p, st, first):
        accv = st["accv"]
        for d in range(4):
            t = 4 * J + d
            kind, at16, c0, sbase, sw = st["at16"][d]
            for s in range(2):
                h = 2 * p + s
                if kind == "packed":
                    mov = at16[:, s, sbase:sbase + sw]
                else:
                    mov = at16[:, s, c0:512]
                nc.tensor.matmul(
                    accv[s][0:128, c0:512],
                    v128_16[:, t, h * 128:h * 128 + 128],
                    mov,
                    start=(first and d == 0), stop=False,
                    skip_group_check=True,
                )

    def att_av(J, p, weave=None, chunk_tail=None, part=None):
        if CFG["skip_attention"] or CFG["skip_vproj"] or CFG["skip_qkproj"]:
            return
        if CFG["skip_av"]:
            return
        st = ATT[(J, p)]
        q_lo = st["q_lo"]
        diag_first = st.get("diag_first", False)
        if part != "diag":
            acc0 = psum.tile([128, 512], F32, name="ps_acc", tag="acc", bufs=2)
            acc1 = psum.tile([128, 512], F32, name="ps_acc", tag="acc", bufs=2)
            st["accv"] = (acc0, acc1)
            if diag_first:
                _diag_av(J, p, st, first=True)
        accv = st["accv"]
        split_last = (CFG.get("split_last_av", False) and part == "off"
                      and J == 3 and p == 1)
        for m in (range(2 * J) if part != "diag" else []):
            at8, sch = st["at8"][m]
            first_m = m == 0 and J > 0 and not diag_first
            for s in range(2):
                h = 2 * p + s
                if sch:
                    for half in range(2):
                        t = 2 * m + half
                        nc.tensor.matmul(
                            accv[s][0:128, 0:512],
                            v128_16[:, t, h * 128:h * 128 + 128],
                            at8[:, half, s * 512:(s + 1) * 512],
                            start=(first_m and half == 0), stop=False,
                            skip_group_check=True,
                        )
                elif split_last and m == 2 * J - 1:
                    # last pair of the last block: per-half non-DR matmuls so
                    # the final AV waits only on the LAST half's exp, not both
                    for half in range(2):
                        t = 2 * m + half
                        nc.tensor.matmul(
                            accv[s][0:128, 0:512],
                            v128_8[:, t, h * 136:h * 136 + 128],
                            at8[:, half, s * 512:(s + 1) * 512],
                            start=False, stop=False,
                            skip_group_check=True,
                        )
                else:
                    nc.tensor.matmul(
                        accv[s][0:128, 0:512],
                        v128_8[:, 2 * m:2 * m + 2, h * 136:h * 136 + 128],
                        at8[:, 0:2, s * 512:(s + 1) * 512],
                        start=first_m, stop=False,
                        perf_mode=DR, skip_group_check=True,
                    )
            _weave(weave)
        if part == "off":
            return
        if part == "diag" and chunk_tail is not None and not CFG["skip_norm"]:
            # pipelined last block: the diag AV already ran in the "off"
            # call (diag_first); here just normalize and ship per q-chunk.
            # accv is fully final by now, so normalize in 256-col groups
            # (halves the DVE recip/mul instruction count at the tail).
            if not diag_first:
                _diag_av(J, p, st, first=(J == 0))
            if CFG.get("tail_recip_hoist", False):
                # chunk 0 keeps narrow recips (fastest pipeline start);
                # chunks 1-3 share one wide recip per head, cutting the DVE
                # stream that paces the tail by ~0.5us
                dns = []
                for s in range(2):
                    dn = sbp.tile([64, 512], F32, name="t_dn", tag="dn",
                                  bufs=4)
                    nc.vector.reciprocal(dn[:, 0:128], accv[s][64:128, 0:128])
                    dns.append(dn)
                for d in range(4):
                    a, b = d * 128, (d + 1) * 128
                    for s in range(2):
                        h = 2 * p + s
                        po = (h % 2) * 64
                        nc.vector.tensor_mul(
                            saT_sb[po:po + 64, h // 2, J * QB + a:(J * QB) + b],
                            accv[s][0:64, a:b], dns[s][:, a:b])
                    chunk_tail(d)
                    if d == 0:
                        for s in range(2):
                            nc.vector.reciprocal(
                                dns[s][:, 128:512], accv[s][64:128, 128:512])
                return
            gw = CFG.get("tail_norm_w", 128)
            for g in range(512 // gw):
                a, b = g * gw, (g + 1) * gw
                for s in range(2):
                    h = 2 * p + s
                    dn = sbp.tile([64, 512], F32, name="t_dn", tag="dn",
                                  bufs=4)[:, 0:gw]
                    nc.vector.reciprocal(dn[:], accv[s][64:128, a:b])
                    po = (h % 2) * 64
                    nc.vector.tensor_mul(
                        saT_sb[po:po + 64, h // 2, J * QB + a:(J * QB) + b],
                        accv[s][0:64, a:b], dn[:])
                for d in range(a // 128, b // 128):
                    chunk_tail(d)
            return
        if not diag_first:
            _diag_av(J, p, st, first=(J == 0))
        if CFG["skip_norm"]:
            return
        # den rows 64:128 of accv (replicated by the ones block of v128):
        # one DVE reciprocal straight off the replicated PSUM rows (same
        # free-size cost as the old [1,512] version, but no Pool
        # partition_broadcast needed), then a DVE multiply.
        dns = []
        for s in range(2):
            dn = sbp.tile([64, 512], F32, name="t_dn", tag="dn",
                          bufs=4)[:, 0:512 - q_lo]
            nc.vector.reciprocal(dn[:], accv[s][64:128, q_lo:512])
            dns.append(dn)
        def norm_cols(a, b):
            for s in range(2):
                h = 2 * p + s
                po = (h % 2) * 64
                nc.vector.tensor_mul(
                    saT_sb[po:po + 64, h // 2, J * QB + a:(J * QB) + b],
                    accv[s][0:64, a:b], dns[s][:, a - q_lo:b - q_lo])
        if chunk_tail is None:
            norm_cols(q_lo, 512)
        else:
            # last block: normalize per 128-col chunk and ship each out slab
            # as soon as its columns are ready
            for q in range(4):
                norm_cols(q * 128, (q + 1) * 128)
                chunk_tail(q)

    def out_slab(J, nqs, copy_on_act=False, engines=None, one_dma=False):
        if CFG["skip_outproj"] or CFG["skip_attention"] or CFG["skip_vproj"] or CFG["skip_qkproj"]:
            return
        r0 = J * QB + nqs * 128
        oe = sbp.tile([128, 1024], BF16 if CFG["out_bf16"] else F32,
                      name="t_oe", tag="oe", bufs=CFG["oe_bufs"])
        for dh in range(2):
            op = psum.tile([128, 512], F32, name="ps_op", tag="op", bufs=2)
            for kt2 in range(2):
                nc.tensor.matmul(
                    op[:],
                    saT_sb[:, kt2, r0:r0 + 128],
                    wo_sb[:, kt2, dh * 512:(dh + 1) * 512],
                    start=(kt2 == 0), stop=(kt2 == 1),
                )
            if engines is not None:
                on_act = engines[dh] == "act"
            else:
                on_act = (copy_on_act and dh == 0) or (J, nqs) in CFG["oe_act"]
            if on_act:
                # tail: Act is past its last exp and otherwise idle; splitting
                # the two copies across Act/DVE lets them overlap
                nc.scalar.copy(oe[:, dh * 512:(dh + 1) * 512], op[:])
            else:
                nc.vector.tensor_copy(oe[:, dh * 512:(dh + 1) * 512], op[:])
            if copy_on_act and not one_dma:
                # ship each half as soon as it is staged; a DVE-staged half
                # is issued from DVE's own DGE queue (in-order after the
                # copy, no extra sem) to take load off the serialized SP
                # HWDGE train at the tail
                dmae = nc.gpsimd if (not on_act and CFG.get("dve_self_dma", False)) else nc.sync
                dmae.dma_start(out[r0:r0 + 128, dh * 512:(dh + 1) * 512],
                               oe[:, dh * 512:(dh + 1) * 512])
        if not copy_on_act or one_dma:
            nc.sync.dma_start(out[r0:r0 + 128, :], oe[:])

    def _qk_evac(nb, ct, dst, src):
        if (nb, ct) in CFG["qk_act"]:
            nc.scalar.activation(
                dst, src, mybir.ActivationFunctionType.Identity,
                bias=bqk8_sb[:, ct:ct + 1])
        else:
            nc.vector.tensor_scalar_add(dst, src, bqk8_sb[:, ct:ct + 1])

    def qk_proj_ct(nb, ct):
        if CFG["skip_qkproj"]:
            return
        ps = psum.tile([128, 512], F32, name="ps_pj", tag="op", bufs=2)
        for j in range(4):
            nc.tensor.matmul(
                ps[:], wqk8_sb[:, 2 * j:2 * j + 2, ct * 128:(ct + 1) * 128],
                xT8_sb[:, 2 * j:2 * j + 2, nb * QB:(nb + 1) * QB],
                start=(j == 0), stop=(j == 3), perf_mode=DR,
            )
        _qk_evac(nb, ct, qkT8_sb[:, ct, nb * QB:(nb + 1) * QB], ps[:])

    # ---- emission order: scores stream ahead of everything so the Act
    # engine (the wall) is continuously fed; av/proj/out work fills PE ----
    for _rep in range(repeat):
        qk_proj(0)
        att_scores(0, 0, ds=[0])
        att_scores(0, 1, ds=[0])
        qk_proj(0, phase="b")
        att_scores(0, 0, ds=[1])
        att_scores(0, 1, ds=[1])
        for ct in range(4):
            qk_proj_ct(1, ct)
        att_scores(0, 0, ds=[2, 3])
        att_scores(0, 1, ds=[2, 3])
        att_scores(1, 0)
        att_scores(1, 1, weave=[lambda nt=nt: v_proj(nt) for nt in range(4)])
        for ct in range(4):
            qk_proj_ct(2, ct)
        att_scores(2, 0, weave=(
            [lambda nt=nt: v_proj(nt) for nt in range(4, 8)] + [prot_qk]))
        att_av(0, 0)
        att_av(0, 1)
        att_av(1, 0)
        att_av(1, 1)
        att_scores(2, 1, weave=(
            [prot_attn] + [lambda nt=nt: v_proj(nt) for nt in range(8, 12)]))
        for ct in range(4):
            qk_proj_ct(3, ct)
        att_scores(3, 0, weave=(
            [lambda nt=nt: v_proj(nt) for nt in range(12, 16)]
            + [lambda q=q: out_slab(0, q) for q in range(4)]))
        att_av(2, 0)
        att_av(2, 1)
        att_scores(3, 1, weave=(
            [lambda q=q: out_slab(1, q) for q in range(4)]
            + [lambda q=q: out_slab(2, q) for q in range(4)]),
                   diag_first=True)
        att_av(3, 0)
        att_av(3, 1, part="off")
        tail_eng = CFG.get("tail_eng", [("act", "act"), ("act", "act"),
                                        ("act", "dve"), ("act", "dve")])
        if CFG.get("tail_pre", False):
            # split the tail out-projection: the kt2=0 half depends only on
            # av(3,0) (long done) — precompute it into the now-idle sc-pool
            # PSUMs before the last exp, so each chunk's critical path has
            # only ONE matmul after its normalize.
            def out_tail_pre(q):
                r0 = 3 * QB + q * 128
                ps = psum.tile([128, 1024], F32, name="ps_ot", tag="sc",
                               bufs=CFG["sc_bufs"])
                for dh in range(2):
                    nc.tensor.matmul(
                        ps[:, dh * 512:(dh + 1) * 512],
                        saT_sb[:, 0, r0:r0 + 128],
                        wo_sb[:, 0, dh * 512:(dh + 1) * 512],
                        start=True, stop=False, skip_group_check=True)
                return ps

            def out_tail_fin(q, ps, engines):
                r0 = 3 * QB + q * 128
                oe = sbp.tile([128, 1024], BF16 if CFG["out_bf16"] else F32,
                              name="t_oe", tag="oe", bufs=CFG["oe_bufs"])
                for dh in range(2):
                    sl = slice(dh * 512, (dh + 1) * 512)
                    nc.tensor.matmul(
                        ps[:, sl],
                        saT_sb[:, 1, r0:r0 + 128],
                        wo_sb[:, 1, sl],
                        start=False, stop=True, skip_group_check=True)
                    if engines[dh] == "act":
                        nc.scalar.copy(oe[:, sl], ps[:, sl])
                    else:
                        nc.vector.tensor_copy(oe[:, sl], ps[:, sl])
                    nc.sync.dma_start(out[r0:r0 + 128, sl], oe[:, sl])

            pres = {0: out_tail_pre(0), 1: out_tail_pre(1)}

            def tail_chunk(q):
                # only the first two chunks are precomputed (sc pool has 2
                # bufs; a third pre would serialize PE behind chunk copies)
                if q in pres:
                    out_tail_fin(q, pres[q], tail_eng[q])
                else:
                    out_slab(3, q, copy_on_act=True, engines=tail_eng[q],
                             one_dma=CFG.get("tail_one_dma", False))

            att_av(3, 1, part="diag", chunk_tail=tail_chunk)
        elif CFG.get("tail_pre0", True):
            # chunk-0-only precompute via the OP pool: its bufs are free
            # after out_slab(2,*) (~81us, BEFORE the last exp), so the
            # kt2=0 matmuls of the first tail chunk run off the critical
            # path; chunks 1-3 keep the old op-pool flow (their WAR on
            # chunk 0's copies matches the status quo).
            r00 = 3 * QB
            pre0 = []
            for dh in range(2):
                ps = psum.tile([128, 512], F32, name="ps_op", tag="op", bufs=2)
                nc.tensor.matmul(
                    ps[:], saT_sb[:, 0, r00:r00 + 128],
                    wo_sb[:, 0, dh * 512:(dh + 1) * 512],
                    start=True, stop=False, skip_group_check=True)
                pre0.append(ps)
            pair_dma = CFG.get("tail_pair_dma", True)
            if pair_dma:
                # persistent staging for the 4 tail slabs: ship slab pairs
                # as ONE dma each (2 HWDGE issues instead of 8 at the tail)
                oe_tail = persist.tile([128, 4, 1024],
                                       BF16 if CFG["out_bf16"] else F32)

            def tail_chunk0(q):
                r0 = 3 * QB + q * 128
                if pair_dma:
                    oe = oe_tail[:, q, :]
                else:
                    oe = sbp.tile([128, 1024],
                                  BF16 if CFG["out_bf16"] else F32,
                                  name="t_oe", tag="oe", bufs=CFG["oe_bufs"])
                paired = pair_dma and q in (0, 1)
                for dh in range(2):
                    sl = slice(dh * 512, (dh + 1) * 512)
                    if q == 0:
                        ps = pre0[dh]
                        nc.tensor.matmul(
                            ps[:], saT_sb[:, 1, r0:r0 + 128],
                            wo_sb[:, 1, sl],
                            start=False, stop=True, skip_group_check=True)
                    else:
                        ps = psum.tile([128, 512], F32, name="ps_op",
                                       tag="op", bufs=2)
                        for kt2 in range(2):
                            nc.tensor.matmul(
                                ps[:], saT_sb[:, kt2, r0:r0 + 128],
                                wo_sb[:, kt2, sl],
                                start=(kt2 == 0), stop=(kt2 == 1))
                    dst = oe_tail[:, q, sl] if paired else oe[:, sl]
                    if tail_eng[q][dh] == "act":
                        nc.scalar.copy(dst, ps[:])
                    else:
                        nc.vector.tensor_copy(dst, ps[:])
                    if not paired:
                        # last slabs ship per half for the earliest finish
                        nc.sync.dma_start(out[r0:r0 + 128, sl], oe[:, sl])
                if paired and q == 1:
                    lo = 3 * QB
                    nc.sync.dma_start(
                        out[lo:lo + 256, :].rearrange("(a p) d -> p a d", a=2),
                        oe_tail[:, 0:2, :])

            att_av(3, 1, part="diag", chunk_tail=tail_chunk0)
        else:
            att_av(3, 1, part="diag",
                   chunk_tail=lambda q: out_slab(
                       3, q, copy_on_act=True, engines=tail_eng[q],
                       one_dma=CFG.get("tail_one_dma", False)))


def build(repeat=1):
    nc = bacc.Bacc("TRN2", target_bir_lowering=False, debug=False,
                   num_devices=N_CORES)
    xT8 = nc.dram_tensor("xT8", [D, N], FP8, kind="ExternalInput").ap()
    xT16 = nc.dram_tensor("xT16", [D, N], BF16, kind="ExternalInput").ap()
    wqk8 = nc.dram_tensor("wqk8", [D, 512], FP8, kind="ExternalInput").ap()
    wqk16 = nc.dram_tensor("wqk16", [D, 512], BF16, kind="ExternalInput").ap()
    wv = nc.dram_tensor("wv", [D, 256], BF16, kind="ExternalInput").ap()
    bqk8 = nc.dram_tensor("bqk8", [4, 128], F32, kind="ExternalInput").ap()
    bqk16 = nc.dram_tensor("bqk16", [4, 128], F32, kind="ExternalInput").ap()
    bv = nc.dram_tensor("bv", [1, 256], F32R, kind="ExternalInput").ap()
    wo = nc.dram_tensor("wo", [256, 1024], F32R, kind="ExternalInput").ap()
    tri16 = nc.dram_tensor("tri16", [128, 128], BF16, kind="ExternalInput").ap()
    out = nc.dram_tensor("out", [N, D], BF16 if CFG["out_bf16"] else F32,
                         kind="ExternalOutput").ap()

    with tile.TileContext(nc) as tc:
        with ExitStack() as ctx:
            _emit(nc, tc, ctx, (xT8, xT16, wqk8, wqk16, wv, bqk8, bqk16, bv,
                                wo, tri16, out), repeat=repeat)
    nc.compile()
    return nc


def make_in_maps(x, Wqkv, bqkv, Wo):
    """Host-side sharding: per-core input dicts."""
    x = np.asarray(x, dtype=np.float32)
    Wqkv = np.asarray(Wqkv, dtype=np.float32)
    bqkv = np.asarray(bqkv, dtype=np.float32)
    Wo = np.asarray(Wo, dtype=np.float32)
    tri16 = np.triu(np.ones((128, 128), np.float32)).astype(ml_dtypes.bfloat16)
    in_maps = []
    for c in range(N_CORES):
        b, g = divmod(c, 4)
        hs = [4 * g + i for i in range(LH)]
        # source chunk order in Wqkv[h] columns: k (0:64), q (64:128), v (128:192)
        # quad32 layout: ct groups {k-half0, k-half1, q-half0, q-half1};
        # within a group, col 32h+j is head hs[h]'s dim j of that half.
        cols8, bias8 = [], []
        for base in (0, 32, 64, 96):  # k0, k1, q0, q1 halves
            blk = np.concatenate(
                [Wqkv[h][:, base:base + 32] for h in hs], axis=1)
            cols8.append(blk)
            bias8.append(np.concatenate([bqkv[h][base:base + 32] for h in hs]))
        wqk8 = np.concatenate(cols8, axis=1)
        bqk8 = np.stack(bias8)
        # pair layout for the bf16 protection path: {kp0, qp0, kp1, qp1}
        cols16, bias16 = [], []
        for p in range(2):
            hA, hB = hs[2 * p], hs[2 * p + 1]
            cols16 += [Wqkv[hA][:, 0:64], Wqkv[hB][:, 0:64]]
            bias16.append(np.concatenate([bqkv[hA][0:64], bqkv[hB][0:64]]))
            cols16 += [Wqkv[hA][:, 64:128], Wqkv[hB][:, 64:128]]
            bias16.append(np.concatenate([bqkv[hA][64:128], bqkv[hB][64:128]]))
        wqk16 = np.concatenate(cols16, axis=1)
        bqk16 = np.stack(bias16)
        xT = np.ascontiguousarray(x[b].T)
        in_maps.append({
            "xT8": xT.astype(ml_dtypes.float8_e4m3),
            "xT16": xT.astype(ml_dtypes.bfloat16),
            "wqk8": np.ascontiguousarray(wqk8).astype(ml_dtypes.float8_e4m3),
            "wqk16": np.ascontiguousarray(wqk16).astype(ml_dtypes.bfloat16),
            "wv": np.ascontiguousarray(
                np.concatenate([Wqkv[h][:, 128:192] for h in hs], axis=1)
            ).astype(ml_dtypes.bfloat16),
            "bqk8": np.ascontiguousarray(bqk8),
            "bqk16": np.ascontiguousarray(bqk16),
            "bv": np.ascontiguousarray(
                np.concatenate([bqkv[h][128:192] for h in hs])[None, :]),
            "wo": np.ascontiguousarray(
                np.concatenate([Wo[h * HD:(h + 1) * HD, :] for h in hs], axis=0)),
            "tri16": tri16,
        })
    return in_maps


def kernel(x, Wqkv, bqkv, Wo, bo):
    if "nc" not in _CACHE:
        _CACHE["nc"] = build()
    nc = _CACHE["nc"]
    in_maps = make_in_maps(x, Wqkv, bqkv, Wo)
    res = bass_utils.run_bass_kernel_spmd(
        nc, in_maps, core_ids=list(range(N_CORES)))
    bo = np.asarray(bo, dtype=np.float32)
    full = np.empty((B, N, D), dtype=np.float32)
    for b in range(B):
        acc = res.results[4 * b]["out"].astype(np.float32).copy()
        for g in range(1, 4):
            acc += res.results[4 * b + g]["out"]
        full[b] = acc + bo[None, :]
    return full

